# revision 71
# baseline (speedup 1.0000x reference)
"""GQA multi-head attention (B=2, S=2048, D=2048, 32 q-heads / 8 kv-heads)
on 8 Trainium2 NeuronCores.

Sharding: DP2 x TP4. Core c = (batch b = c//4, group g = c%4). Each core owns
batch b and q-heads 8g..8g+7 (kv heads 2g, 2g+1): Wq col-shard [2048, 512],
Wk/Wv col-shard [2048, 128], Wo row-shard [512, 2048]. Host sums the 4
partial outputs per batch.

Projections run as error-compensated fp8 DoubleRow matmuls: x = hi + lo in
fp8e4m3 (weights prescaled x64), product = hi@Whi + hi@Wlo + lo@Whi — 25%
fewer PE cycles than fp16 at ~1e-3 relative error. Attention core is fp16:
  qh^T [512, S]   (head h at partitions 64*(h%2), slot h//2)
  kh^T parity-duplicated in ktd; V in [k-pos, d] layout + ones column (x64)
  S^T [k, q] -> exp((.)/8/4096) on ACT ([128,1024] tiles), optionally a
  fraction on DVE via Schraudolph fp16-bitcast exp (K_SCHRAU pairs per 8)
  ctx [q, 65] = expS-tile^T @ V_aug (col 64 = x64-scaled softmax denom)
  ctx_n = ctx * recip(denom)  (DVE), PE-transposed into ctxT [128, 4, S]
  out [S, 2048] = ctxT^T @ Wo, interleaved per q-quarter behind the exp wave.
"""

import os as _os
from contextlib import ExitStack

import numpy as np
import ml_dtypes

import jax

try:
    jax.config.update("jax_compilation_cache_dir", "/tmp/jax_bass_cache")
    jax.config.update("jax_persistent_cache_min_compile_time_secs", 1.0)
except Exception:
    pass

from jax.sharding import Mesh, PartitionSpec, NamedSharding
from jax.experimental.shard_map import shard_map

import concourse.bass as bass
import concourse.mybir as mybir
import concourse.tile as tile
from concourse import bacc, bass2jax

F16 = mybir.dt.float16
F32 = mybir.dt.float32
F8 = mybir.dt.float8e4
I16 = mybir.dt.int16
AF = mybir.ActivationFunctionType
ALU = mybir.AluOpType
DR = mybir.MatmulPerfMode.DoubleRow

B, S, DM = 2, 2048, 2048
NHEAD = 8              # q heads per core
NKV = 2                # kv heads per core
DH = 64
DQ = NHEAD * DH        # 512: per-core q-projection width
DT = DM // 128         # 16 contraction tiles
NC = 8
WS = 64.0              # fp8 weight prescale
SCALE = 1.0 / (8.0 * WS * WS)   # exp scale: 1/sqrt(64) / (q,k weight scales)
NKT = S // 128         # 16 key tiles

# exp engine split: number of kt-pairs (of 8) per unit computed on DVE via
# Schraudolph bit-trick instead of ACT. 0 = all ACT.
SCHRAU = int(_os.environ.get("K_SCHRAU", "2"))
SCHRAU_C = float(_os.environ.get("K_SCHRAU_C", "-0.0425"))

_cache = {}


def _emit(ctx, tc, dram):
    nc = tc.nc
    qTh, qTl, kTh, kTl, vTh, vTl = (dram[n] for n in
                                    ("qTh", "qTl", "kTh", "kTl", "vTh", "vTl"))
    wqh, wql, wkh, wkl, wvh, wvl, wo, out = (
        dram[n] for n in ("wqh", "wql", "wkh", "wkl", "wvh", "wvl", "wo", "out"))

    pp = ctx.enter_context(tc.tile_pool(name="persist", bufs=1))
    wqh_sb = pp.tile([128, DT, DQ], F8, tag="wqh")
    wql_sb = pp.tile([128, DT, DQ], F8, tag="wql")
    wkh_sb = pp.tile([128, DT, 2 * DH], F8, tag="wkh")
    wkl_sb = pp.tile([128, DT, 2 * DH], F8, tag="wkl")
    wvh_sb = pp.tile([128, DT, 2 * DH], F8, tag="wvh")
    wvl_sb = pp.tile([128, DT, 2 * DH], F8, tag="wvl")
    wo_sb = pp.tile([128, 4, DM], F16, tag="wo")
    # head h at partitions 64*(h%2); slot = h//2
    qtp = pp.tile([128, 4, S], F16, tag="qtp")
    # kh^T parity-duplicated: ktd[p, kv, k] = kh[kv, p % 64, k]
    ktd = pp.tile([128, NKV, S], F16, tag="ktd")
    # V + ones column; vsb[kpos, kt, kv, 0:64] = 64*vh, [.., 64] = 64.0
    vsb = pp.tile([128, NKT, NKV, DH + 1], F16, tag="vsb")
    ctxT = pp.tile([128, 4, S], F16, tag="ctxT")

    # DMA priority: wk + kT chunks feed the scores critical path first.
    nc.sync.dma_start(wkh_sb[:], wkh.rearrange("(dt p) m -> p dt m", p=128))
    nc.sync.dma_start(wkl_sb[:], wkl.rearrange("(dt p) m -> p dt m", p=128))
    nc.gpsimd.memset(vsb[:, :, :, DH:DH + 1], WS)

    stage = ctx.enter_context(tc.tile_pool(name="stage", bufs=2))
    expp = ctx.enter_context(tc.tile_pool(name="expp", bufs=2))
    smal = ctx.enter_context(tc.tile_pool(name="small", bufs=2))
    outp = ctx.enter_context(tc.tile_pool(name="outp", bufs=2))
    psum = ctx.enter_context(tc.tile_pool(name="psum", bufs=1, space="PSUM"))

    def comp_dr_mms(p_out, wh_sb, wl_sb, ch, col0, ncols, rcols):
        """3-term compensated fp8 DoubleRow matmul group into psum p_out.

        lhsT terms: (wh, wh, wl) sliced [:, 2d:2d+2, col0:col0+ncols];
        rhs terms:  (ch_h, ch_h, ch_l) sliced [:, 2d:2d+2, 0:rcols].
        """
        ch_h, ch_l = ch
        terms = ((wh_sb, ch_h), (wl_sb, ch_h), (wh_sb, ch_l))
        n = DT // 2
        for ti, (w_sb, c_sb) in enumerate(terms):
            for d in range(n):
                nc.tensor.matmul(
                    p_out, w_sb[:, 2 * d:2 * d + 2, col0:col0 + ncols],
                    c_sb[:, 2 * d:2 * d + 2, 0:rcols],
                    start=(ti == 0 and d == 0),
                    stop=(ti == 2 and d == n - 1), perf_mode=DR)

    def dma_chunk(src_h, src_l, so, ncols, nm):
        h = stage.tile([128, DT, ncols], F8, tag="instage", bufs=5,
                       name=f"{nm}_h")
        l = stage.tile([128, DT, ncols], F8, tag="instage", bufs=5,
                       name=f"{nm}_l")
        nc.sync.dma_start(
            h[:], src_h.rearrange("(dt p) s -> p dt s", p=128)[:, :, so:so + ncols])
        nc.sync.dma_start(
            l[:], src_l.rearrange("(dt p) s -> p dt s", p=128)[:, :, so:so + ncols])
        return h, l

    # ---------------- projections ----------------
    def kproj_chunk(kc):
        return dma_chunk(kTh, kTl, kc * 512, 512, f"k_ch_{kc}")

    def emit_kproj_mm(kc, ch):
        so = kc * 512
        pk = psum.tile([128, 512], F32, tag="mm", bufs=1, name=f"pk_{kc}")
        comp_dr_mms(pk[:], wkh_sb, wkl_sb, ch, 0, 2 * DH, 512)
        # rows 0:64 = kv0, 64:128 = kv1 (natural Wk column order)
        nc.vector.tensor_copy(ktd[0:64, 0, so:so + 512], pk[0:64, :])
        nc.vector.tensor_copy(ktd[64:128, 1, so:so + 512], pk[64:128, :])
        # parity fixups: duplicate each kv head to the other 64 partitions
        nc.sync.dma_start(ktd[64:128, 0, so:so + 512], ktd[0:64, 0, so:so + 512])
        nc.sync.dma_start(ktd[0:64, 1, so:so + 512], ktd[64:128, 1, so:so + 512])

    def emit_kproj(kc):
        emit_kproj_mm(kc, kproj_chunk(kc))

    def qproj_chunk(qc):
        return dma_chunk(qTh, qTl, qc * 512, 512, f"q_ch_{qc}")

    def emit_qproj_t(qc, ch, t):
        so = qc * 512
        pq = psum.tile([128, 512], F32, tag="mm", bufs=1, name=f"pq_{t}_{qc}")
        comp_dr_mms(pq[:], wqh_sb, wql_sb, ch, t * 128, 128, 512)
        nc.vector.tensor_copy(qtp[:, t, so:so + 512], pq[:])

    def emit_qproj(qc):
        """All 4 head-pair tiles for one q-quarter from a single qT chunk."""
        ch = qproj_chunk(qc)
        for t in range(4):
            emit_qproj_t(qc, ch, t)

    def vproj_chunk(vc):
        return dma_chunk(vTh, vTl, vc * 512, 512, f"v_ch_{vc}")

    def emit_vproj_i(vc, chp, i):
        vh_ch, vl_ch = chp
        if True:
            kt = vc * 4 + i
            pv = psum.tile([128, 512], F32, tag="mm", bufs=1, name=f"pv_{kt}")
            terms = ((vh_ch, wvh_sb), (vh_ch, wvl_sb), (vl_ch, wvh_sb))
            for ti, (v_sb, w_sb) in enumerate(terms):
                for d in range(DT // 2):
                    nc.tensor.matmul(
                        pv[:, 0:128],
                        v_sb[:, 2 * d:2 * d + 2, i * 128:(i + 1) * 128],
                        w_sb[:, 2 * d:2 * d + 2, :],
                        start=(ti == 0 and d == 0),
                        stop=(ti == 2 and d == DT // 2 - 1), perf_mode=DR)
            nc.vector.tensor_copy(vsb[:, kt, :, 0:DH], pv[:, 0:128])

    def emit_vproj(vc):
        chp = vproj_chunk(vc)
        for i in range(4):
            emit_vproj_i(vc, chp, i)

    # ---------------- attention units ----------------
    sch_A = SCALE * np.log2(np.e) * 1024.0
    sch_B = (15.0 + SCHRAU_C) * 1024.0

    def emit_score_pair(h, qq, kt2, exps):
        """One kt-pair of score matmuls + its exp into the exps slot."""
        par = 64 * (h % 2)
        kv = h // 4
        qoff = qq * 512
        ps = psum.tile([128, 2, 512], F32, tag="sc", bufs=3,
                       name=f"ps_{h}_{qq}_{kt2}")
        for j in range(2):
            kt = 2 * kt2 + j
            nc.tensor.matmul(
                ps[:, j, :],
                ktd[par:par + 64, kv, kt * 128:(kt + 1) * 128],
                qtp[par:par + 64, h // 2, qoff:qoff + 512])
        dst = exps[:, 2 * kt2:2 * kt2 + 2, :]
        if kt2 in (3, 7)[:SCHRAU] if SCHRAU <= 2 else kt2 >= NKT // 2 - SCHRAU:
            # Schraudolph: fp16 bits ~= round(x*log2e*1024 + (15+c)*1024)
            nc.vector.tensor_scalar(dst.bitcast(I16), ps[:],
                                    sch_A, sch_B, ALU.mult, ALU.add)
        else:
            nc.scalar.activation(dst, ps[:], AF.Exp, scale=SCALE)

    def emit_ctx_qt(h, qq, exps, cxp, qt, half):
        """Half of a qt ctx accumulation group (8 matmuls, 65 cols)."""
        kv = h // 4
        for kt in range(half * 8, half * 8 + 8):
            nc.tensor.matmul(
                cxp[:, qt, 0:DH + 1], exps[:, kt, qt * 128:(qt + 1) * 128],
                vsb[:, kt, kv, :], start=(kt == 0), stop=(kt == NKT - 1))

    def emit_norm(h, qq, cxp, css):
        """Normalize into the head-pair css tile (even: cols 0:64, odd: 64:128)."""
        co = DH * (h % 2)
        rc = smal.tile([128, 4], F32, tag="recip", bufs=3, name=f"rc_{h}_{qq}")
        nc.vector.reciprocal(rc[:], cxp[:, :, DH:DH + 1])
        for qt in range(4):
            # Pool/GPSIMD cannot read PSUM on HW; DVE does the normalize.
            nc.vector.tensor_scalar(css[:, qt, co:co + DH], cxp[:, qt, 0:DH],
                                    rc[:, qt:qt + 1], None, ALU.mult)

    def emit_transp_qt(h, qq, css, qt):
        """XBAR DMA-transpose of a head-pair's [128q, 128d] css qt into ctxT."""
        nc.sync.dma_start_transpose(
            ctxT[:, h // 2, qq * 512 + qt * 128:qq * 512 + (qt + 1) * 128],
            css[:, qt, :])

    ost_cur = [None]
    eng_alt = [0]

    def emit_outc_chunk(qt, ch):
        """One phase-C psum group: out rows [qt*128, +128), cols [ch*512, +512)."""
        if ch == 0:
            ost_cur[0] = outp.tile([128, DM], F16, tag="ostage", bufs=3,
                                   name=f"ost_{qt}")
        ost = ost_cur[0]
        po2 = psum.tile([128, 2, 512], F32, tag="sc", bufs=3,
                        name=f"po_{qt}_{ch}")
        po = po2[:, 0, :]
        for i in range(4):
            nc.tensor.matmul(po[:], ctxT[:, i, qt * 128:(qt + 1) * 128],
                             wo_sb[:, i, ch * 512:(ch + 1) * 512],
                             start=(i == 0), stop=(i == 3))
        dst = ost[:, ch * 512:(ch + 1) * 512]
        if eng_alt[0] % 2 == 0:
            nc.vector.tensor_copy(dst, po)
        else:
            nc.scalar.copy(dst, po)
        eng_alt[0] += 1
        nc.sync.dma_start(out[qt * 128:(qt + 1) * 128, ch * 512:(ch + 1) * 512],
                          ost[:, ch * 512:(ch + 1) * 512])

    # ---------------- schedule ----------------
    # Startup: kc0 -> Q weights + first qT quarter -> kc1-3. Q proj runs
    # while the remaining kT chunks stream.
    emit_kproj(0)
    wqr_h = wqh.rearrange("(dt p) m -> p dt m", p=128)
    wqr_l = wql.rearrange("(dt p) m -> p dt m", p=128)
    nc.sync.dma_start(wqh_sb[:, :, 0:128], wqr_h[:, :, 0:128])
    nc.sync.dma_start(wql_sb[:, :, 0:128], wqr_l[:, :, 0:128])
    q0ch = qproj_chunk(0)
    emit_qproj_t(0, q0ch, 0)
    nc.sync.dma_start(wqh_sb[:, :, 128:512], wqr_h[:, :, 128:512])
    nc.sync.dma_start(wql_sb[:, :, 128:512], wqr_l[:, :, 128:512])
    for t in range(1, 4):
        emit_qproj_t(0, q0ch, t)
    for kc in range(1, 4):
        emit_kproj(kc)
    nc.sync.dma_start(wvh_sb[:], wvh.rearrange("(dt p) m -> p dt m", p=128))
    nc.sync.dma_start(wvl_sb[:], wvl.rearrange("(dt p) m -> p dt m", p=128))

    units = [(h, qq) for qq in range(4) for h in range(NHEAD)]

    def emit_wo_dma():
        nc.sync.dma_start(wo_sb[:], wo.rearrange("(i p) d -> p i d", p=128))

    big_fill = []
    for vc in range(4):
        big_fill.append(lambda vc=vc: chunks.__setitem__(f"v{vc}", vproj_chunk(vc)))
        for i in range(4):
            big_fill.append(lambda vc=vc, i=i: emit_vproj_i(vc, chunks[f"v{vc}"], i))
    for qc in range(1, 4):
        big_fill.append(lambda qc=qc: chunks.__setitem__(f"q{qc}", qproj_chunk(qc)))
        for t in range(4):
            big_fill.append(lambda qc=qc, t=t: emit_qproj_t(qc, chunks[f"q{qc}"], t))
        if qc == 1:
            big_fill.append(emit_wo_dma)
    chunks = {}

    # Per-unit state threading: fillers from unit u-1 are woven between unit
    # u's score pairs so PE has work while ACT drains the exp queue.
    fillers = []          # queue of small PE-work closures
    big_i = [0]
    unit_no = [0]
    pend_exps = None
    css_prev = [None]

    def make_unit_fillers(ph, pqq, pexps):
        # after the projections drain (~unit 16) the mm bank is idle:
        # alternate ctx tiles between cx and mm for a 2-deep ring.
        unit_no[0] += 1
        if unit_no[0] > 16 and unit_no[0] % 2 == 0:
            cxp = psum.tile([128, 4, 128], F32, tag="mm", bufs=1,
                            name=f"cxp_{ph}_{pqq}")
        else:
            cxp = psum.tile([128, 4, 128], F32, tag="cx", bufs=1,
                            name=f"cxp_{ph}_{pqq}")
        if ph % 2 == 0:
            css = smal.tile([128, 4, 2 * DH], F16, tag="css", bufs=3,
                            name=f"css_{ph}_{pqq}")
            css_prev[0] = css
        else:
            css = css_prev[0]
        fs = []
        # qt-major: each qt's psum accumulation group must run start->stop
        # before the next qt's start (start marks the whole 2KB zero-region
        # of the shared cxp bank pending-zero, wiping in-flight partials).
        for qt in range(4):
            for half in range(2):
                fs.append(lambda qt=qt, half=half:
                          emit_ctx_qt(ph, pqq, pexps, cxp, qt, half))
        fs.append(lambda: emit_norm(ph, pqq, cxp, css))
        if ph % 2 == 1:
            for qt in range(4):
                fs.append(lambda qt=qt: emit_transp_qt(ph, pqq, css, qt))
        if ph == NHEAD - 1:
            for qt in range(pqq * 4, pqq * 4 + 4):
                for ch in range(4):
                    fs.append(lambda qt=qt, ch=ch: emit_outc_chunk(qt, ch))
        return fs

    for u, (h, qq) in enumerate(units):
        exps = expp.tile([128, NKT, 512], F16, tag="exp", bufs=4,
                         name=f"exp_{h}_{qq}")
        for _ in range(2):
            if big_i[0] < len(big_fill):
                fillers.append(big_fill[big_i[0]])
                big_i[0] += 1
        if pend_exps is not None:
            fillers.extend(make_unit_fillers(*pend_exps))
        for kt2 in range(NKT // 2):
            emit_score_pair(h, qq, kt2, exps)
            # weave fillers between score matmuls; drain backlog smoothly
            budget = 1 if len(fillers) <= (NKT // 2 - 1 - kt2) else 2
            for _ in range(min(budget, len(fillers))):
                fillers.pop(0)()
        pend_exps = (h, qq, exps)

    fillers.extend(make_unit_fillers(*pend_exps))
    for f in fillers:
        f()


def _build():
    nc = bacc.Bacc("TRN2", target_bir_lowering=False, debug=False,
                   num_devices=NC)
    dram = {}
    for nm in ("qTh", "qTl", "kTh", "kTl", "vTh", "vTl"):
        dram[nm] = nc.dram_tensor(nm, [DM, S], F8, kind="ExternalInput").ap()
    dram["wqh"] = nc.dram_tensor("wqh", [DM, DQ], F8, kind="ExternalInput").ap()
    dram["wql"] = nc.dram_tensor("wql", [DM, DQ], F8, kind="ExternalInput").ap()
    for nm in ("wkh", "wkl", "wvh", "wvl"):
        dram[nm] = nc.dram_tensor(nm, [DM, 2 * DH], F8,
                                  kind="ExternalInput").ap()
    dram["wo"] = nc.dram_tensor("wo", [DQ, DM], F16, kind="ExternalInput").ap()
    dram["out"] = nc.dram_tensor("out", [S, DM], F16, kind="ExternalOutput").ap()
    with tile.TileContext(nc) as tc:
        with ExitStack() as ctx:
            _emit(ctx, tc, dram)
    nc.compile()
    return nc


def _make_runner(nc, n_cores=NC):
    """Build the sharded jit callable once; reuse across kernel() calls."""
    bass2jax.install_neuronx_cc_hook()
    partition_name = nc.partition_id_tensor.name if nc.partition_id_tensor else None
    in_names, out_names, out_avals, zero_outs = [], [], [], []
    for alloc in nc.m.functions[0].allocations:
        if not isinstance(alloc, mybir.MemoryLocationSet):
            continue
        name = alloc.memorylocations[0].name
        if alloc.kind == "ExternalInput":
            if name != partition_name:
                in_names.append(name)
        elif alloc.kind == "ExternalOutput":
            out_names.append(name)
            shape = tuple(alloc.tensor_shape)
            dtype = mybir.dt.np(alloc.dtype)
            out_avals.append(jax.core.ShapedArray(shape, dtype))
            zero_outs.append(np.zeros(shape, dtype))
    n_params = len(in_names)
    n_outs = len(out_avals)
    in_names_all = in_names + out_names
    if partition_name is not None:
        in_names_all.append(partition_name)
    donate = tuple(range(n_params, n_params + n_outs))

    def _body(*args):
        operands = list(args)
        if partition_name is not None:
            operands.append(bass2jax.partition_id_tensor())
        outs = bass2jax._bass_exec_p.bind(
            *operands,
            out_avals=tuple(out_avals),
            in_names=tuple(in_names_all),
            out_names=tuple(out_names),
            lowering_input_output_aliases=(),
            sim_require_finite=True,
            sim_require_nnan=True,
            nc=nc,
        )
        return tuple(outs)

    devices = jax.devices()[:n_cores]
    mesh = Mesh(np.asarray(devices), ("core",))
    in_specs = (PartitionSpec("core"),) * (n_params + n_outs)
    out_specs = (PartitionSpec("core"),) * len(out_names)
    sharded = jax.jit(
        shard_map(_body, mesh=mesh, in_specs=in_specs, out_specs=out_specs,
                  check_rep=False),
        donate_argnums=donate, keep_unused=True)
    sh = NamedSharding(mesh, PartitionSpec("core"))
    return sharded, in_names, out_names, zero_outs, sh


def _run(in_maps):
    if "nc" not in _cache:
        _cache["nc"] = _build()
    if "runner" not in _cache:
        _cache["runner"] = _make_runner(_cache["nc"])
    sharded, in_names, out_names, zero_outs, sh = _cache["runner"]
    n = NC
    concat_in = [
        jax.device_put(
            np.concatenate([np.asarray(in_maps[c][nm]) for c in range(n)], 0), sh)
        for nm in in_names
    ]
    zeros = [
        jax.device_put(np.zeros((n * z.shape[0], *z.shape[1:]), z.dtype), sh)
        for z in zero_outs
    ]
    outs = sharded(*concat_in, *zeros)
    i = out_names.index("out")
    arr = np.asarray(outs[i])           # [NC*S, DM]
    return arr.reshape(n, S, DM)


E4 = ml_dtypes.float8_e4m3


def _hilo(x):
    h = x.astype(E4)
    l = (x - h.astype(np.float32)).astype(E4)
    return np.ascontiguousarray(h), np.ascontiguousarray(l)


def kernel(q, k, v, Wq, Wk, Wv, Wo):
    q = np.asarray(q, dtype=np.float32)
    k = np.asarray(k, dtype=np.float32)
    v = np.asarray(v, dtype=np.float32)
    qT = [_hilo(q[b].T) for b in range(B)]
    kT = [_hilo(k[b].T) for b in range(B)]
    vT = [_hilo(v[b].T) for b in range(B)]
    Wq64 = np.asarray(Wq, dtype=np.float32) * WS
    Wk64 = np.asarray(Wk, dtype=np.float32) * WS
    Wv64 = np.asarray(Wv, dtype=np.float32) * WS
    Wo6 = np.asarray(Wo, dtype=np.float32).astype(np.float16)

    in_maps = []
    for c in range(NC):
        b, g = c // 4, c % 4
        wqh, wql = _hilo(Wq64[:, g * DQ:(g + 1) * DQ])
        wkh, wkl = _hilo(Wk64[:, g * 2 * DH:(g + 1) * 2 * DH])
        wvh, wvl = _hilo(Wv64[:, g * 2 * DH:(g + 1) * 2 * DH])
        in_maps.append({
            "qTh": qT[b][0], "qTl": qT[b][1],
            "kTh": kT[b][0], "kTl": kT[b][1],
            "vTh": vT[b][0], "vTl": vT[b][1],
            "wqh": wqh, "wql": wql, "wkh": wkh, "wkl": wkl,
            "wvh": wvh, "wvl": wvl,
            "wo": np.ascontiguousarray(Wo6[g * DQ:(g + 1) * DQ, :]),
        })
    partials = _run(in_maps)            # [8, S, DM]
    out = partials.astype(np.float32, copy=False).reshape(B, 4, S, DM).sum(1)
    return out


# revision 72
# speedup vs baseline: 1.0069x; 1.0069x over previous
"""GQA multi-head attention (B=2, S=2048, D=2048, 32 q-heads / 8 kv-heads)
on 8 Trainium2 NeuronCores.

Sharding: DP2 x TP4. Core c = (batch b = c//4, group g = c%4). Each core owns
batch b and q-heads 8g..8g+7 (kv heads 2g, 2g+1): Wq col-shard [2048, 512],
Wk/Wv col-shard [2048, 128], Wo row-shard [512, 2048]. Host sums the 4
partial outputs per batch.

Projections run as error-compensated fp8 DoubleRow matmuls: x = hi + lo in
fp8e4m3 (weights prescaled x64), product = hi@Whi + hi@Wlo + lo@Whi — 25%
fewer PE cycles than fp16 at ~1e-3 relative error. Attention core is fp16:
  qh^T [512, S]   (head h at partitions 64*(h%2), slot h//2)
  kh^T parity-duplicated in ktd; V in [k-pos, d] layout + ones column (x64)
  S^T [k, q] -> exp((.)/8/4096) on ACT ([128,1024] tiles), optionally a
  fraction on DVE via Schraudolph fp16-bitcast exp (K_SCHRAU pairs per 8)
  ctx [q, 65] = expS-tile^T @ V_aug (col 64 = x64-scaled softmax denom)
  ctx_n = ctx * recip(denom)  (DVE), PE-transposed into ctxT [128, 4, S]
  out [S, 2048] = ctxT^T @ Wo, interleaved per q-quarter behind the exp wave.
"""

import os as _os
from contextlib import ExitStack

import numpy as np
import ml_dtypes

import jax

try:
    jax.config.update("jax_compilation_cache_dir", "/tmp/jax_bass_cache")
    jax.config.update("jax_persistent_cache_min_compile_time_secs", 1.0)
except Exception:
    pass

from jax.sharding import Mesh, PartitionSpec, NamedSharding
from jax.experimental.shard_map import shard_map

import concourse.bass as bass
import concourse.mybir as mybir
import concourse.tile as tile
from concourse import bacc, bass2jax

F16 = mybir.dt.float16
F32 = mybir.dt.float32
F8 = mybir.dt.float8e4
I16 = mybir.dt.int16
AF = mybir.ActivationFunctionType
ALU = mybir.AluOpType
DR = mybir.MatmulPerfMode.DoubleRow

B, S, DM = 2, 2048, 2048
NHEAD = 8              # q heads per core
NKV = 2                # kv heads per core
DH = 64
DQ = NHEAD * DH        # 512: per-core q-projection width
DT = DM // 128         # 16 contraction tiles
NC = 8
WS = 64.0              # fp8 weight prescale
SCALE = 1.0 / (8.0 * WS * WS)   # exp scale: 1/sqrt(64) / (q,k weight scales)
NKT = S // 128         # 16 key tiles

# exp engine split: number of kt-pairs (of 8) per unit computed on DVE via
# Schraudolph bit-trick instead of ACT. 0 = all ACT.
SCHRAU = int(_os.environ.get("K_SCHRAU", "2"))
SCHRAU_C = float(_os.environ.get("K_SCHRAU_C", "-0.0425"))

_cache = {}


def _emit(ctx, tc, dram):
    nc = tc.nc
    qTh, qTl, kTh, kTl, vTh, vTl = (dram[n] for n in
                                    ("qTh", "qTl", "kTh", "kTl", "vTh", "vTl"))
    wqh, wql, wkh, wkl, wvh, wvl, wo, out = (
        dram[n] for n in ("wqh", "wql", "wkh", "wkl", "wvh", "wvl", "wo", "out"))

    pp = ctx.enter_context(tc.tile_pool(name="persist", bufs=1))
    wqh_sb = pp.tile([128, DT, DQ], F8, tag="wqh")
    wql_sb = pp.tile([128, DT, DQ], F8, tag="wql")
    wkh_sb = pp.tile([128, DT, 2 * DH], F8, tag="wkh")
    wkl_sb = pp.tile([128, DT, 2 * DH], F8, tag="wkl")
    wvh_sb = pp.tile([128, DT, 2 * DH], F8, tag="wvh")
    wvl_sb = pp.tile([128, DT, 2 * DH], F8, tag="wvl")
    wo_sb = pp.tile([128, 4, DM], F16, tag="wo")
    # head h at partitions 64*(h%2); slot = h//2
    qtp = pp.tile([128, 4, S], F16, tag="qtp")
    # kh^T parity-duplicated: ktd[p, kv, k] = kh[kv, p % 64, k]
    ktd = pp.tile([128, NKV, S], F16, tag="ktd")
    # V + ones column; vsb[kpos, kt, kv, 0:64] = 64*vh, [.., 64] = 64.0
    vsb = pp.tile([128, NKT, NKV, DH + 1], F16, tag="vsb")
    ctxT = pp.tile([128, 4, S], F16, tag="ctxT")

    # DMA priority: wk + kT chunks feed the scores critical path first.
    nc.sync.dma_start(wkh_sb[:], wkh.rearrange("(dt p) m -> p dt m", p=128))
    nc.sync.dma_start(wkl_sb[:], wkl.rearrange("(dt p) m -> p dt m", p=128))
    nc.gpsimd.memset(vsb[:, :, :, DH:DH + 1], WS)

    stage = ctx.enter_context(tc.tile_pool(name="stage", bufs=2))
    expp = ctx.enter_context(tc.tile_pool(name="expp", bufs=2))
    smal = ctx.enter_context(tc.tile_pool(name="small", bufs=2))
    outp = ctx.enter_context(tc.tile_pool(name="outp", bufs=2))
    psum = ctx.enter_context(tc.tile_pool(name="psum", bufs=1, space="PSUM"))

    def comp_dr_mms(p_out, wh_sb, wl_sb, ch, col0, ncols, rcols):
        """3-term compensated fp8 DoubleRow matmul group into psum p_out.

        lhsT terms: (wh, wh, wl) sliced [:, 2d:2d+2, col0:col0+ncols];
        rhs terms:  (ch_h, ch_h, ch_l) sliced [:, 2d:2d+2, 0:rcols].
        """
        ch_h, ch_l = ch
        terms = ((wh_sb, ch_h), (wl_sb, ch_h), (wh_sb, ch_l))
        n = DT // 2
        for ti, (w_sb, c_sb) in enumerate(terms):
            for d in range(n):
                nc.tensor.matmul(
                    p_out, w_sb[:, 2 * d:2 * d + 2, col0:col0 + ncols],
                    c_sb[:, 2 * d:2 * d + 2, 0:rcols],
                    start=(ti == 0 and d == 0),
                    stop=(ti == 2 and d == n - 1), perf_mode=DR)

    def dma_chunk(src_h, src_l, so, ncols, nm):
        h = stage.tile([128, DT, ncols], F8, tag="instage", bufs=5,
                       name=f"{nm}_h")
        l = stage.tile([128, DT, ncols], F8, tag="instage", bufs=5,
                       name=f"{nm}_l")
        nc.sync.dma_start(
            h[:], src_h.rearrange("(dt p) s -> p dt s", p=128)[:, :, so:so + ncols])
        nc.sync.dma_start(
            l[:], src_l.rearrange("(dt p) s -> p dt s", p=128)[:, :, so:so + ncols])
        return h, l

    # ---------------- projections ----------------
    def kproj_chunk(kc):
        return dma_chunk(kTh, kTl, kc * 512, 512, f"k_ch_{kc}")

    def emit_kproj_mm(kc, ch):
        so = kc * 512
        pk = psum.tile([128, 512], F32, tag="mm", bufs=1, name=f"pk_{kc}")
        comp_dr_mms(pk[:], wkh_sb, wkl_sb, ch, 0, 2 * DH, 512)
        # rows 0:64 = kv0, 64:128 = kv1 (natural Wk column order)
        nc.vector.tensor_copy(ktd[0:64, 0, so:so + 512], pk[0:64, :])
        nc.vector.tensor_copy(ktd[64:128, 1, so:so + 512], pk[64:128, :])
        # parity fixups: duplicate each kv head to the other 64 partitions
        nc.sync.dma_start(ktd[64:128, 0, so:so + 512], ktd[0:64, 0, so:so + 512])
        nc.sync.dma_start(ktd[0:64, 1, so:so + 512], ktd[64:128, 1, so:so + 512])

    def emit_kproj(kc):
        emit_kproj_mm(kc, kproj_chunk(kc))

    def qproj_chunk(qc):
        return dma_chunk(qTh, qTl, qc * 512, 512, f"q_ch_{qc}")

    def emit_qproj_t(qc, ch, t):
        so = qc * 512
        pq = psum.tile([128, 512], F32, tag="mm", bufs=1, name=f"pq_{t}_{qc}")
        comp_dr_mms(pq[:], wqh_sb, wql_sb, ch, t * 128, 128, 512)
        nc.vector.tensor_copy(qtp[:, t, so:so + 512], pq[:])

    def emit_qproj(qc):
        """All 4 head-pair tiles for one q-quarter from a single qT chunk."""
        ch = qproj_chunk(qc)
        for t in range(4):
            emit_qproj_t(qc, ch, t)

    def vproj_chunk(vc):
        return dma_chunk(vTh, vTl, vc * 512, 512, f"v_ch_{vc}")

    def emit_vproj_i(vc, chp, i):
        vh_ch, vl_ch = chp
        if True:
            kt = vc * 4 + i
            pv = psum.tile([128, 512], F32, tag="mm", bufs=1, name=f"pv_{kt}")
            terms = ((vh_ch, wvh_sb), (vh_ch, wvl_sb), (vl_ch, wvh_sb))
            for ti, (v_sb, w_sb) in enumerate(terms):
                for d in range(DT // 2):
                    nc.tensor.matmul(
                        pv[:, 0:128],
                        v_sb[:, 2 * d:2 * d + 2, i * 128:(i + 1) * 128],
                        w_sb[:, 2 * d:2 * d + 2, :],
                        start=(ti == 0 and d == 0),
                        stop=(ti == 2 and d == DT // 2 - 1), perf_mode=DR)
            nc.vector.tensor_copy(vsb[:, kt, :, 0:DH], pv[:, 0:128])

    def emit_vproj(vc):
        chp = vproj_chunk(vc)
        for i in range(4):
            emit_vproj_i(vc, chp, i)

    # ---------------- attention units ----------------
    sch_A = SCALE * np.log2(np.e) * 1024.0
    sch_B = (15.0 + SCHRAU_C) * 1024.0

    def emit_score_pair(h, qq, kt2, exps):
        """One kt-pair of score matmuls + its exp into the exps slot."""
        par = 64 * (h % 2)
        kv = h // 4
        qoff = qq * 512
        ps = psum.tile([128, 2, 512], F32, tag="sc", bufs=3,
                       name=f"ps_{h}_{qq}_{kt2}")
        for j in range(2):
            kt = 2 * kt2 + j
            nc.tensor.matmul(
                ps[:, j, :],
                ktd[par:par + 64, kv, kt * 128:(kt + 1) * 128],
                qtp[par:par + 64, h // 2, qoff:qoff + 512])
        dst = exps[:, 2 * kt2:2 * kt2 + 2, :]
        if kt2 in (3, 7)[:SCHRAU] if SCHRAU <= 2 else kt2 >= NKT // 2 - SCHRAU:
            # Schraudolph: fp16 bits ~= round(x*log2e*1024 + (15+c)*1024)
            nc.vector.tensor_scalar(dst.bitcast(I16), ps[:],
                                    sch_A, sch_B, ALU.mult, ALU.add)
        else:
            nc.scalar.activation(dst, ps[:], AF.Exp, scale=SCALE)

    def emit_ctx_qt(h, qq, exps, cxp, qt, half):
        """Half of a qt ctx accumulation group (8 matmuls, 65 cols)."""
        kv = h // 4
        for kt in range(half * 8, half * 8 + 8):
            nc.tensor.matmul(
                cxp[:, qt, 0:DH + 1], exps[:, kt, qt * 128:(qt + 1) * 128],
                vsb[:, kt, kv, :], start=(kt == 0), stop=(kt == NKT - 1))

    def emit_norm(h, qq, cxp, css):
        """Normalize into the head-pair css tile (even: cols 0:64, odd: 64:128)."""
        co = DH * (h % 2)
        rc = smal.tile([128, 4], F32, tag="recip", bufs=3, name=f"rc_{h}_{qq}")
        nc.vector.reciprocal(rc[:], cxp[:, :, DH:DH + 1])
        for qt in range(4):
            # Pool/GPSIMD cannot read PSUM on HW; DVE does the normalize.
            nc.vector.tensor_scalar(css[:, qt, co:co + DH], cxp[:, qt, 0:DH],
                                    rc[:, qt:qt + 1], None, ALU.mult)

    def emit_transp_qt(h, qq, css, qt):
        """XBAR DMA-transpose of a head-pair's [128q, 128d] css qt into ctxT."""
        nc.sync.dma_start_transpose(
            ctxT[:, h // 2, qq * 512 + qt * 128:qq * 512 + (qt + 1) * 128],
            css[:, qt, :])

    ost_cur = [None]
    eng_alt = [0]

    def emit_outc_chunk(qt, ch):
        """One phase-C psum group: out rows [qt*128, +128), cols [ch*512, +512)."""
        if ch == 0:
            ost_cur[0] = outp.tile([128, DM], F16, tag="ostage", bufs=3,
                                   name=f"ost_{qt}")
        ost = ost_cur[0]
        po2 = psum.tile([128, 2, 512], F32, tag="sc", bufs=3,
                        name=f"po_{qt}_{ch}")
        po = po2[:, 0, :]
        for i in range(4):
            nc.tensor.matmul(po[:], ctxT[:, i, qt * 128:(qt + 1) * 128],
                             wo_sb[:, i, ch * 512:(ch + 1) * 512],
                             start=(i == 0), stop=(i == 3))
        dst = ost[:, ch * 512:(ch + 1) * 512]
        if eng_alt[0] % 2 == 0:
            nc.vector.tensor_copy(dst, po)
        else:
            nc.scalar.copy(dst, po)
        eng_alt[0] += 1
        nc.sync.dma_start(out[qt * 128:(qt + 1) * 128, ch * 512:(ch + 1) * 512],
                          ost[:, ch * 512:(ch + 1) * 512])

    # ---------------- schedule ----------------
    # Startup: kc0 -> Q weights + first qT quarter -> kc1-3. Q proj runs
    # while the remaining kT chunks stream.
    emit_kproj(0)
    wqr_h = wqh.rearrange("(dt p) m -> p dt m", p=128)
    wqr_l = wql.rearrange("(dt p) m -> p dt m", p=128)
    nc.sync.dma_start(wqh_sb[:, :, 0:128], wqr_h[:, :, 0:128])
    nc.sync.dma_start(wql_sb[:, :, 0:128], wqr_l[:, :, 0:128])
    q0ch = qproj_chunk(0)
    emit_qproj_t(0, q0ch, 0)
    nc.sync.dma_start(wqh_sb[:, :, 128:512], wqr_h[:, :, 128:512])
    nc.sync.dma_start(wql_sb[:, :, 128:512], wqr_l[:, :, 128:512])
    for t in range(1, 4):
        emit_qproj_t(0, q0ch, t)
    for kc in range(1, 4):
        emit_kproj(kc)
    nc.sync.dma_start(wvh_sb[:], wvh.rearrange("(dt p) m -> p dt m", p=128))
    nc.sync.dma_start(wvl_sb[:], wvl.rearrange("(dt p) m -> p dt m", p=128))

    units = [(h, qq) for qq in range(4) for h in range(NHEAD)]

    def emit_wo_dma():
        nc.sync.dma_start(wo_sb[:], wo.rearrange("(i p) d -> p i d", p=128))

    big_fill = []
    for vc in range(4):
        big_fill.append(lambda vc=vc: chunks.__setitem__(f"v{vc}", vproj_chunk(vc)))
        for i in range(4):
            big_fill.append(lambda vc=vc, i=i: emit_vproj_i(vc, chunks[f"v{vc}"], i))
    for qc in range(1, 4):
        big_fill.append(lambda qc=qc: chunks.__setitem__(f"q{qc}", qproj_chunk(qc)))
        for t in range(4):
            big_fill.append(lambda qc=qc, t=t: emit_qproj_t(qc, chunks[f"q{qc}"], t))
        if qc == 1:
            big_fill.append(emit_wo_dma)
    chunks = {}

    # Per-unit state threading: fillers from unit u-1 are woven between unit
    # u's score pairs so PE has work while ACT drains the exp queue.
    fillers = []          # queue of small PE-work closures
    big_i = [0]
    unit_no = [0]
    pend_q = []           # lag-2: process fillers two units behind
    css_prev = [None]

    def make_unit_fillers(ph, pqq, pexps):
        # after the projections drain (~unit 16) the mm bank is idle:
        # alternate ctx tiles between cx and mm for a 2-deep ring.
        unit_no[0] += 1
        if unit_no[0] > 16 and unit_no[0] % 2 == 0:
            cxp = psum.tile([128, 4, 128], F32, tag="mm", bufs=1,
                            name=f"cxp_{ph}_{pqq}")
        else:
            cxp = psum.tile([128, 4, 128], F32, tag="cx", bufs=1,
                            name=f"cxp_{ph}_{pqq}")
        if ph % 2 == 0:
            css = smal.tile([128, 4, 2 * DH], F16, tag="css", bufs=3,
                            name=f"css_{ph}_{pqq}")
            css_prev[0] = css
        else:
            css = css_prev[0]
        fs = []
        # qt-major: each qt's psum accumulation group must run start->stop
        # before the next qt's start (start marks the whole 2KB zero-region
        # of the shared cxp bank pending-zero, wiping in-flight partials).
        for qt in range(4):
            for half in range(2):
                fs.append(lambda qt=qt, half=half:
                          emit_ctx_qt(ph, pqq, pexps, cxp, qt, half))
        fs.append(lambda: emit_norm(ph, pqq, cxp, css))
        if ph % 2 == 1:
            for qt in range(4):
                fs.append(lambda qt=qt: emit_transp_qt(ph, pqq, css, qt))
        if ph == NHEAD - 1:
            for qt in range(pqq * 4, pqq * 4 + 4):
                for ch in range(4):
                    fs.append(lambda qt=qt, ch=ch: emit_outc_chunk(qt, ch))
        return fs

    for u, (h, qq) in enumerate(units):
        exps = expp.tile([128, NKT, 512], F16, tag="exp", bufs=4,
                         name=f"exp_{h}_{qq}")
        for _ in range(2):
            if big_i[0] < len(big_fill):
                fillers.append(big_fill[big_i[0]])
                big_i[0] += 1
        if len(pend_q) >= 2:
            fillers.extend(make_unit_fillers(*pend_q.pop(0)))
        for kt2 in range(NKT // 2):
            emit_score_pair(h, qq, kt2, exps)
            # weave fillers between score matmuls; drain backlog smoothly
            budget = 1 if len(fillers) <= (NKT // 2 - 1 - kt2) else 2
            for _ in range(min(budget, len(fillers))):
                fillers.pop(0)()
        pend_q.append((h, qq, exps))

    for p in pend_q:
        fillers.extend(make_unit_fillers(*p))
    for f in fillers:
        f()


def _build():
    nc = bacc.Bacc("TRN2", target_bir_lowering=False, debug=False,
                   num_devices=NC)
    dram = {}
    for nm in ("qTh", "qTl", "kTh", "kTl", "vTh", "vTl"):
        dram[nm] = nc.dram_tensor(nm, [DM, S], F8, kind="ExternalInput").ap()
    dram["wqh"] = nc.dram_tensor("wqh", [DM, DQ], F8, kind="ExternalInput").ap()
    dram["wql"] = nc.dram_tensor("wql", [DM, DQ], F8, kind="ExternalInput").ap()
    for nm in ("wkh", "wkl", "wvh", "wvl"):
        dram[nm] = nc.dram_tensor(nm, [DM, 2 * DH], F8,
                                  kind="ExternalInput").ap()
    dram["wo"] = nc.dram_tensor("wo", [DQ, DM], F16, kind="ExternalInput").ap()
    dram["out"] = nc.dram_tensor("out", [S, DM], F16, kind="ExternalOutput").ap()
    with tile.TileContext(nc) as tc:
        with ExitStack() as ctx:
            _emit(ctx, tc, dram)
    nc.compile()
    return nc


def _make_runner(nc, n_cores=NC):
    """Build the sharded jit callable once; reuse across kernel() calls."""
    bass2jax.install_neuronx_cc_hook()
    partition_name = nc.partition_id_tensor.name if nc.partition_id_tensor else None
    in_names, out_names, out_avals, zero_outs = [], [], [], []
    for alloc in nc.m.functions[0].allocations:
        if not isinstance(alloc, mybir.MemoryLocationSet):
            continue
        name = alloc.memorylocations[0].name
        if alloc.kind == "ExternalInput":
            if name != partition_name:
                in_names.append(name)
        elif alloc.kind == "ExternalOutput":
            out_names.append(name)
            shape = tuple(alloc.tensor_shape)
            dtype = mybir.dt.np(alloc.dtype)
            out_avals.append(jax.core.ShapedArray(shape, dtype))
            zero_outs.append(np.zeros(shape, dtype))
    n_params = len(in_names)
    n_outs = len(out_avals)
    in_names_all = in_names + out_names
    if partition_name is not None:
        in_names_all.append(partition_name)
    donate = tuple(range(n_params, n_params + n_outs))

    def _body(*args):
        operands = list(args)
        if partition_name is not None:
            operands.append(bass2jax.partition_id_tensor())
        outs = bass2jax._bass_exec_p.bind(
            *operands,
            out_avals=tuple(out_avals),
            in_names=tuple(in_names_all),
            out_names=tuple(out_names),
            lowering_input_output_aliases=(),
            sim_require_finite=True,
            sim_require_nnan=True,
            nc=nc,
        )
        return tuple(outs)

    devices = jax.devices()[:n_cores]
    mesh = Mesh(np.asarray(devices), ("core",))
    in_specs = (PartitionSpec("core"),) * (n_params + n_outs)
    out_specs = (PartitionSpec("core"),) * len(out_names)
    sharded = jax.jit(
        shard_map(_body, mesh=mesh, in_specs=in_specs, out_specs=out_specs,
                  check_rep=False),
        donate_argnums=donate, keep_unused=True)
    sh = NamedSharding(mesh, PartitionSpec("core"))
    return sharded, in_names, out_names, zero_outs, sh


def _run(in_maps):
    if "nc" not in _cache:
        _cache["nc"] = _build()
    if "runner" not in _cache:
        _cache["runner"] = _make_runner(_cache["nc"])
    sharded, in_names, out_names, zero_outs, sh = _cache["runner"]
    n = NC
    concat_in = [
        jax.device_put(
            np.concatenate([np.asarray(in_maps[c][nm]) for c in range(n)], 0), sh)
        for nm in in_names
    ]
    zeros = [
        jax.device_put(np.zeros((n * z.shape[0], *z.shape[1:]), z.dtype), sh)
        for z in zero_outs
    ]
    outs = sharded(*concat_in, *zeros)
    i = out_names.index("out")
    arr = np.asarray(outs[i])           # [NC*S, DM]
    return arr.reshape(n, S, DM)


E4 = ml_dtypes.float8_e4m3


def _hilo(x):
    h = x.astype(E4)
    l = (x - h.astype(np.float32)).astype(E4)
    return np.ascontiguousarray(h), np.ascontiguousarray(l)


def kernel(q, k, v, Wq, Wk, Wv, Wo):
    q = np.asarray(q, dtype=np.float32)
    k = np.asarray(k, dtype=np.float32)
    v = np.asarray(v, dtype=np.float32)
    qT = [_hilo(q[b].T) for b in range(B)]
    kT = [_hilo(k[b].T) for b in range(B)]
    vT = [_hilo(v[b].T) for b in range(B)]
    Wq64 = np.asarray(Wq, dtype=np.float32) * WS
    Wk64 = np.asarray(Wk, dtype=np.float32) * WS
    Wv64 = np.asarray(Wv, dtype=np.float32) * WS
    Wo6 = np.asarray(Wo, dtype=np.float32).astype(np.float16)

    in_maps = []
    for c in range(NC):
        b, g = c // 4, c % 4
        wqh, wql = _hilo(Wq64[:, g * DQ:(g + 1) * DQ])
        wkh, wkl = _hilo(Wk64[:, g * 2 * DH:(g + 1) * 2 * DH])
        wvh, wvl = _hilo(Wv64[:, g * 2 * DH:(g + 1) * 2 * DH])
        in_maps.append({
            "qTh": qT[b][0], "qTl": qT[b][1],
            "kTh": kT[b][0], "kTl": kT[b][1],
            "vTh": vT[b][0], "vTl": vT[b][1],
            "wqh": wqh, "wql": wql, "wkh": wkh, "wkl": wkl,
            "wvh": wvh, "wvl": wvl,
            "wo": np.ascontiguousarray(Wo6[g * DQ:(g + 1) * DQ, :]),
        })
    partials = _run(in_maps)            # [8, S, DM]
    out = partials.astype(np.float32, copy=False).reshape(B, 4, S, DM).sum(1)
    return out


# revision 78
# speedup vs baseline: 1.0073x; 1.0004x over previous
"""GQA multi-head attention (B=2, S=2048, D=2048, 32 q-heads / 8 kv-heads)
on 8 Trainium2 NeuronCores.

Sharding: DP2 x TP4. Core c = (batch b = c//4, group g = c%4). Each core owns
batch b and q-heads 8g..8g+7 (kv heads 2g, 2g+1): Wq col-shard [2048, 512],
Wk/Wv col-shard [2048, 128], Wo row-shard [512, 2048]. Host sums the 4
partial outputs per batch.

Projections run as error-compensated fp8 DoubleRow matmuls: x = hi + lo in
fp8e4m3 (weights prescaled x64), product = hi@Whi + hi@Wlo + lo@Whi — 25%
fewer PE cycles than fp16 at ~1e-3 relative error. Attention core is fp16:
  qh^T [512, S]   (head h at partitions 64*(h%2), slot h//2)
  kh^T parity-duplicated in ktd; V in [k-pos, d] layout + ones column (x64)
  S^T [k, q] -> exp((.)/8/4096) on ACT ([128,1024] tiles), optionally a
  fraction on DVE via Schraudolph fp16-bitcast exp (K_SCHRAU pairs per 8)
  ctx [q, 65] = expS-tile^T @ V_aug (col 64 = x64-scaled softmax denom)
  ctx_n = ctx * recip(denom)  (DVE), PE-transposed into ctxT [128, 4, S]
  out [S, 2048] = ctxT^T @ Wo, interleaved per q-quarter behind the exp wave.
"""

import os as _os
from contextlib import ExitStack

import numpy as np
import ml_dtypes

import jax

try:
    jax.config.update("jax_compilation_cache_dir", "/tmp/jax_bass_cache")
    jax.config.update("jax_persistent_cache_min_compile_time_secs", 1.0)
except Exception:
    pass

from jax.sharding import Mesh, PartitionSpec, NamedSharding
from jax.experimental.shard_map import shard_map

import concourse.bass as bass
import concourse.mybir as mybir
import concourse.tile as tile
from concourse import bacc, bass2jax

F16 = mybir.dt.float16
F32 = mybir.dt.float32
F8 = mybir.dt.float8e4
I16 = mybir.dt.int16
AF = mybir.ActivationFunctionType
ALU = mybir.AluOpType
DR = mybir.MatmulPerfMode.DoubleRow

B, S, DM = 2, 2048, 2048
NHEAD = 8              # q heads per core
NKV = 2                # kv heads per core
DH = 64
DQ = NHEAD * DH        # 512: per-core q-projection width
DT = DM // 128         # 16 contraction tiles
NC = 8
WS = 64.0              # fp8 weight prescale
SCALE = 1.0 / (8.0 * WS * WS)   # exp scale: 1/sqrt(64) / (q,k weight scales)
NKT = S // 128         # 16 key tiles

# exp engine split: number of kt-pairs (of 8) per unit computed on DVE via
# Schraudolph bit-trick instead of ACT. 0 = all ACT.
SCHRAU = int(_os.environ.get("K_SCHRAU", "2"))
SCHRAU_C = float(_os.environ.get("K_SCHRAU_C", "-0.0425"))

_cache = {}


def _emit(ctx, tc, dram):
    nc = tc.nc
    qTh, qTl, kTh, kTl, vTh, vTl = (dram[n] for n in
                                    ("qTh", "qTl", "kTh", "kTl", "vTh", "vTl"))
    wqh, wql, wkh, wkl, wvh, wvl, wo, out = (
        dram[n] for n in ("wqh", "wql", "wkh", "wkl", "wvh", "wvl", "wo", "out"))

    pp = ctx.enter_context(tc.tile_pool(name="persist", bufs=1))
    wqh_sb = pp.tile([128, DT, DQ], F8, tag="wqh")
    wql_sb = pp.tile([128, DT, DQ], F8, tag="wql")
    wkh_sb = pp.tile([128, DT, 2 * DH], F8, tag="wkh")
    wkl_sb = pp.tile([128, DT, 2 * DH], F8, tag="wkl")
    wvh_sb = pp.tile([128, DT, 2 * DH], F8, tag="wvh")
    wvl_sb = pp.tile([128, DT, 2 * DH], F8, tag="wvl")
    wo_sb = pp.tile([128, 4, DM], F16, tag="wo")
    # head h at partitions 64*(h%2); slot = h//2
    qtp = pp.tile([128, 4, S], F16, tag="qtp")
    # kh^T parity-duplicated: ktd[p, kv, k] = kh[kv, p % 64, k]
    ktd = pp.tile([128, NKV, S], F16, tag="ktd")
    # V + ones column; vsb[kpos, kt, kv, 0:64] = 64*vh, [.., 64] = 64.0
    vsb = pp.tile([128, NKT, NKV, DH + 1], F16, tag="vsb")
    ctxT = pp.tile([128, 4, S], F16, tag="ctxT")

    # DMA priority: wk + kT chunks feed the scores critical path first.
    nc.sync.dma_start(wkh_sb[:], wkh.rearrange("(dt p) m -> p dt m", p=128))
    nc.sync.dma_start(wkl_sb[:], wkl.rearrange("(dt p) m -> p dt m", p=128))
    nc.gpsimd.memset(vsb[:, :, :, DH:DH + 1], WS)

    stage = ctx.enter_context(tc.tile_pool(name="stage", bufs=2))
    expp = ctx.enter_context(tc.tile_pool(name="expp", bufs=2))
    smal = ctx.enter_context(tc.tile_pool(name="small", bufs=2))
    outp = ctx.enter_context(tc.tile_pool(name="outp", bufs=2))
    psum = ctx.enter_context(tc.tile_pool(name="psum", bufs=1, space="PSUM"))

    def comp_dr_mms(p_out, wh_sb, wl_sb, ch, col0, ncols, rcols):
        """3-term compensated fp8 DoubleRow matmul group into psum p_out.

        lhsT terms: (wh, wh, wl) sliced [:, 2d:2d+2, col0:col0+ncols];
        rhs terms:  (ch_h, ch_h, ch_l) sliced [:, 2d:2d+2, 0:rcols].
        """
        ch_h, ch_l = ch
        terms = ((wh_sb, ch_h), (wl_sb, ch_h), (wh_sb, ch_l))
        n = DT // 2
        for ti, (w_sb, c_sb) in enumerate(terms):
            for d in range(n):
                nc.tensor.matmul(
                    p_out, w_sb[:, 2 * d:2 * d + 2, col0:col0 + ncols],
                    c_sb[:, 2 * d:2 * d + 2, 0:rcols],
                    start=(ti == 0 and d == 0),
                    stop=(ti == 2 and d == n - 1), perf_mode=DR)

    def dma_chunk(src_h, src_l, so, ncols, nm):
        h = stage.tile([128, DT, ncols], F8, tag="instage", bufs=5,
                       name=f"{nm}_h")
        l = stage.tile([128, DT, ncols], F8, tag="instage", bufs=5,
                       name=f"{nm}_l")
        nc.sync.dma_start(
            h[:], src_h.rearrange("(dt p) s -> p dt s", p=128)[:, :, so:so + ncols])
        nc.sync.dma_start(
            l[:], src_l.rearrange("(dt p) s -> p dt s", p=128)[:, :, so:so + ncols])
        return h, l

    # ---------------- projections ----------------
    def kproj_chunk(kc):
        return dma_chunk(kTh, kTl, kc * 512, 512, f"k_ch_{kc}")

    def emit_kproj_mm(kc, ch):
        so = kc * 512
        pk = psum.tile([128, 512], F32, tag="mm", bufs=1, name=f"pk_{kc}")
        comp_dr_mms(pk[:], wkh_sb, wkl_sb, ch, 0, 2 * DH, 512)
        # rows 0:64 = kv0, 64:128 = kv1 (natural Wk column order)
        nc.vector.tensor_copy(ktd[0:64, 0, so:so + 512], pk[0:64, :])
        nc.vector.tensor_copy(ktd[64:128, 1, so:so + 512], pk[64:128, :])
        # parity fixups: duplicate each kv head to the other 64 partitions
        nc.sync.dma_start(ktd[64:128, 0, so:so + 512], ktd[0:64, 0, so:so + 512])
        nc.sync.dma_start(ktd[0:64, 1, so:so + 512], ktd[64:128, 1, so:so + 512])

    def emit_kproj(kc):
        emit_kproj_mm(kc, kproj_chunk(kc))

    def qproj_chunk(qc):
        return dma_chunk(qTh, qTl, qc * 512, 512, f"q_ch_{qc}")

    def emit_qproj_t(qc, ch, t):
        so = qc * 512
        pq = psum.tile([128, 512], F32, tag="mm", bufs=1, name=f"pq_{t}_{qc}")
        comp_dr_mms(pq[:], wqh_sb, wql_sb, ch, t * 128, 128, 512)
        nc.vector.tensor_copy(qtp[:, t, so:so + 512], pq[:])

    def emit_qproj(qc):
        """All 4 head-pair tiles for one q-quarter from a single qT chunk."""
        ch = qproj_chunk(qc)
        for t in range(4):
            emit_qproj_t(qc, ch, t)

    def vproj_chunk(vc):
        return dma_chunk(vTh, vTl, vc * 512, 512, f"v_ch_{vc}")

    def emit_vproj_i(vc, chp, i):
        vh_ch, vl_ch = chp
        if True:
            kt = vc * 4 + i
            pv = psum.tile([128, 512], F32, tag="mm", bufs=1, name=f"pv_{kt}")
            terms = ((vh_ch, wvh_sb), (vh_ch, wvl_sb), (vl_ch, wvh_sb))
            for ti, (v_sb, w_sb) in enumerate(terms):
                for d in range(DT // 2):
                    nc.tensor.matmul(
                        pv[:, 0:128],
                        v_sb[:, 2 * d:2 * d + 2, i * 128:(i + 1) * 128],
                        w_sb[:, 2 * d:2 * d + 2, :],
                        start=(ti == 0 and d == 0),
                        stop=(ti == 2 and d == DT // 2 - 1), perf_mode=DR)
            nc.vector.tensor_copy(vsb[:, kt, :, 0:DH], pv[:, 0:128])

    def emit_vproj(vc):
        chp = vproj_chunk(vc)
        for i in range(4):
            emit_vproj_i(vc, chp, i)

    # ---------------- attention units ----------------
    sch_A = SCALE * np.log2(np.e) * 1024.0
    sch_B = (15.0 + SCHRAU_C) * 1024.0

    def emit_score_pair(h, qq, kt2, exps):
        """One kt-pair of score matmuls + its exp into the exps slot."""
        par = 64 * (h % 2)
        kv = h // 4
        qoff = qq * 512
        ps = psum.tile([128, 2, 512], F32, tag="sc", bufs=3,
                       name=f"ps_{h}_{qq}_{kt2}")
        for j in range(2):
            kt = 2 * kt2 + j
            nc.tensor.matmul(
                ps[:, j, :],
                ktd[par:par + 64, kv, kt * 128:(kt + 1) * 128],
                qtp[par:par + 64, h // 2, qoff:qoff + 512])
        dst = exps[:, 2 * kt2:2 * kt2 + 2, :]
        if kt2 in (3, 7)[:SCHRAU] if SCHRAU <= 2 else kt2 >= NKT // 2 - SCHRAU:
            # Schraudolph: fp16 bits ~= round(x*log2e*1024 + (15+c)*1024)
            nc.vector.tensor_scalar(dst.bitcast(I16), ps[:],
                                    sch_A, sch_B, ALU.mult, ALU.add)
        else:
            nc.scalar.activation(dst, ps[:], AF.Exp, scale=SCALE)

    def emit_ctx_qt(h, qq, exps, cxp, qt, half):
        """Half of a qt ctx accumulation group (8 matmuls, 65 cols)."""
        kv = h // 4
        for kt in range(half * 8, half * 8 + 8):
            nc.tensor.matmul(
                cxp[:, qt, 0:DH + 1], exps[:, kt, qt * 128:(qt + 1) * 128],
                vsb[:, kt, kv, :], start=(kt == 0), stop=(kt == NKT - 1))

    def emit_norm(h, qq, cxp, css):
        """Normalize into the head-pair css tile (even: cols 0:64, odd: 64:128)."""
        co = DH * (h % 2)
        rc = smal.tile([128, 4], F32, tag="recip", bufs=3, name=f"rc_{h}_{qq}")
        nc.vector.reciprocal(rc[:], cxp[:, :, DH:DH + 1])
        for qt in range(4):
            # Pool/GPSIMD cannot read PSUM on HW; DVE does the normalize.
            nc.vector.tensor_scalar(css[:, qt, co:co + DH], cxp[:, qt, 0:DH],
                                    rc[:, qt:qt + 1], None, ALU.mult)

    def emit_transp_qt(h, qq, css, qt):
        """XBAR DMA-transpose of a head-pair's [128q, 128d] css qt into ctxT."""
        nc.sync.dma_start_transpose(
            ctxT[:, h // 2, qq * 512 + qt * 128:qq * 512 + (qt + 1) * 128],
            css[:, qt, :])

    ost_cur = [None]
    eng_alt = [0]

    def emit_outc_chunk(qt, ch):
        """One phase-C psum group: out rows [qt*128, +128), cols [ch*512, +512)."""
        if ch == 0:
            ost_cur[0] = outp.tile([128, DM], F16, tag="ostage", bufs=3,
                                   name=f"ost_{qt}")
        ost = ost_cur[0]
        po2 = psum.tile([128, 2, 512], F32, tag="sc", bufs=3,
                        name=f"po_{qt}_{ch}")
        po = po2[:, 0, :]
        for i in range(4):
            nc.tensor.matmul(po[:], ctxT[:, i, qt * 128:(qt + 1) * 128],
                             wo_sb[:, i, ch * 512:(ch + 1) * 512],
                             start=(i == 0), stop=(i == 3))
        dst = ost[:, ch * 512:(ch + 1) * 512]
        if eng_alt[0] % 2 == 0:
            nc.vector.tensor_copy(dst, po)
        else:
            nc.scalar.copy(dst, po)
        eng_alt[0] += 1
        nc.sync.dma_start(out[qt * 128:(qt + 1) * 128, ch * 512:(ch + 1) * 512],
                          ost[:, ch * 512:(ch + 1) * 512])

    # ---------------- schedule ----------------
    # Startup: kc0 -> Q weights + first qT quarter -> kc1-3. Q proj runs
    # while the remaining kT chunks stream.
    emit_kproj(0)
    wqr_h = wqh.rearrange("(dt p) m -> p dt m", p=128)
    wqr_l = wql.rearrange("(dt p) m -> p dt m", p=128)
    nc.sync.dma_start(wqh_sb[:, :, 0:128], wqr_h[:, :, 0:128])
    nc.sync.dma_start(wql_sb[:, :, 0:128], wqr_l[:, :, 0:128])
    q0ch = qproj_chunk(0)
    emit_qproj_t(0, q0ch, 0)
    nc.sync.dma_start(wqh_sb[:, :, 128:512], wqr_h[:, :, 128:512])
    nc.sync.dma_start(wql_sb[:, :, 128:512], wqr_l[:, :, 128:512])
    for t in range(1, 4):
        emit_qproj_t(0, q0ch, t)
    for kc in range(1, 4):
        emit_kproj(kc)
    nc.sync.dma_start(wvh_sb[:], wvh.rearrange("(dt p) m -> p dt m", p=128))
    nc.sync.dma_start(wvl_sb[:], wvl.rearrange("(dt p) m -> p dt m", p=128))

    units = [(h, qq) for qq in range(4) for h in range(NHEAD)]

    def emit_wo_dma():
        nc.sync.dma_start(wo_sb[:], wo.rearrange("(i p) d -> p i d", p=128))

    big_fill = []
    for vc in range(4):
        big_fill.append(lambda vc=vc: chunks.__setitem__(f"v{vc}", vproj_chunk(vc)))
        for i in range(4):
            big_fill.append(lambda vc=vc, i=i: emit_vproj_i(vc, chunks[f"v{vc}"], i))
    for qc in range(1, 4):
        big_fill.append(lambda qc=qc: chunks.__setitem__(f"q{qc}", qproj_chunk(qc)))
        for t in range(4):
            big_fill.append(lambda qc=qc, t=t: emit_qproj_t(qc, chunks[f"q{qc}"], t))
        if qc == 1:
            big_fill.append(emit_wo_dma)
    chunks = {}

    # Per-unit state threading: fillers from unit u-1 are woven between unit
    # u's score pairs so PE has work while ACT drains the exp queue.
    fillers = []          # queue of small PE-work closures
    big_i = [0]
    unit_no = [0]
    pend_q = []           # lag-2: process fillers two units behind
    css_prev = [None]

    def make_unit_fillers(ph, pqq, pexps):
        # after the projections drain (~unit 16) the mm bank is idle:
        # alternate ctx tiles between cx and mm for a 2-deep ring.
        unit_no[0] += 1
        if unit_no[0] > 14 and unit_no[0] % 2 == 0:
            cxp = psum.tile([128, 4, 128], F32, tag="mm", bufs=1,
                            name=f"cxp_{ph}_{pqq}")
        else:
            cxp = psum.tile([128, 4, 128], F32, tag="cx", bufs=1,
                            name=f"cxp_{ph}_{pqq}")
        if ph % 2 == 0:
            css = smal.tile([128, 4, 2 * DH], F16, tag="css", bufs=3,
                            name=f"css_{ph}_{pqq}")
            css_prev[0] = css
        else:
            css = css_prev[0]
        fs = []
        # qt-major: each qt's psum accumulation group must run start->stop
        # before the next qt's start (start marks the whole 2KB zero-region
        # of the shared cxp bank pending-zero, wiping in-flight partials).
        for qt in range(4):
            for half in range(2):
                fs.append(lambda qt=qt, half=half:
                          emit_ctx_qt(ph, pqq, pexps, cxp, qt, half))
        fs.append(lambda: emit_norm(ph, pqq, cxp, css))
        if ph % 2 == 1:
            for qt in range(4):
                fs.append(lambda qt=qt: emit_transp_qt(ph, pqq, css, qt))
        if ph == NHEAD - 1:
            for qt in range(pqq * 4, pqq * 4 + 4):
                for ch in range(4):
                    fs.append(lambda qt=qt, ch=ch: emit_outc_chunk(qt, ch))
        return fs

    for u, (h, qq) in enumerate(units):
        exps = expp.tile([128, NKT, 512], F16, tag="exp", bufs=4,
                         name=f"exp_{h}_{qq}")
        for _ in range(2):
            if big_i[0] < len(big_fill):
                fillers.append(big_fill[big_i[0]])
                big_i[0] += 1
        if len(pend_q) >= 2:
            fillers.extend(make_unit_fillers(*pend_q.pop(0)))
        for kt2 in range(NKT // 2):
            emit_score_pair(h, qq, kt2, exps)
            # weave fillers between score matmuls; drain backlog smoothly
            budget = 1 if len(fillers) <= (NKT // 2 - 1 - kt2) else 2
            for _ in range(min(budget, len(fillers))):
                fillers.pop(0)()
        pend_q.append((h, qq, exps))

    for p in pend_q:
        fillers.extend(make_unit_fillers(*p))
    for f in fillers:
        f()


def _build():
    nc = bacc.Bacc("TRN2", target_bir_lowering=False, debug=False,
                   num_devices=NC)
    dram = {}
    for nm in ("qTh", "qTl", "kTh", "kTl", "vTh", "vTl"):
        dram[nm] = nc.dram_tensor(nm, [DM, S], F8, kind="ExternalInput").ap()
    dram["wqh"] = nc.dram_tensor("wqh", [DM, DQ], F8, kind="ExternalInput").ap()
    dram["wql"] = nc.dram_tensor("wql", [DM, DQ], F8, kind="ExternalInput").ap()
    for nm in ("wkh", "wkl", "wvh", "wvl"):
        dram[nm] = nc.dram_tensor(nm, [DM, 2 * DH], F8,
                                  kind="ExternalInput").ap()
    dram["wo"] = nc.dram_tensor("wo", [DQ, DM], F16, kind="ExternalInput").ap()
    dram["out"] = nc.dram_tensor("out", [S, DM], F16, kind="ExternalOutput").ap()
    with tile.TileContext(nc) as tc:
        with ExitStack() as ctx:
            _emit(ctx, tc, dram)
    nc.compile()
    return nc


def _make_runner(nc, n_cores=NC):
    """Build the sharded jit callable once; reuse across kernel() calls."""
    bass2jax.install_neuronx_cc_hook()
    partition_name = nc.partition_id_tensor.name if nc.partition_id_tensor else None
    in_names, out_names, out_avals, zero_outs = [], [], [], []
    for alloc in nc.m.functions[0].allocations:
        if not isinstance(alloc, mybir.MemoryLocationSet):
            continue
        name = alloc.memorylocations[0].name
        if alloc.kind == "ExternalInput":
            if name != partition_name:
                in_names.append(name)
        elif alloc.kind == "ExternalOutput":
            out_names.append(name)
            shape = tuple(alloc.tensor_shape)
            dtype = mybir.dt.np(alloc.dtype)
            out_avals.append(jax.core.ShapedArray(shape, dtype))
            zero_outs.append(np.zeros(shape, dtype))
    n_params = len(in_names)
    n_outs = len(out_avals)
    in_names_all = in_names + out_names
    if partition_name is not None:
        in_names_all.append(partition_name)
    donate = tuple(range(n_params, n_params + n_outs))

    def _body(*args):
        operands = list(args)
        if partition_name is not None:
            operands.append(bass2jax.partition_id_tensor())
        outs = bass2jax._bass_exec_p.bind(
            *operands,
            out_avals=tuple(out_avals),
            in_names=tuple(in_names_all),
            out_names=tuple(out_names),
            lowering_input_output_aliases=(),
            sim_require_finite=True,
            sim_require_nnan=True,
            nc=nc,
        )
        return tuple(outs)

    devices = jax.devices()[:n_cores]
    mesh = Mesh(np.asarray(devices), ("core",))
    in_specs = (PartitionSpec("core"),) * (n_params + n_outs)
    out_specs = (PartitionSpec("core"),) * len(out_names)
    sharded = jax.jit(
        shard_map(_body, mesh=mesh, in_specs=in_specs, out_specs=out_specs,
                  check_rep=False),
        donate_argnums=donate, keep_unused=True)
    sh = NamedSharding(mesh, PartitionSpec("core"))
    return sharded, in_names, out_names, zero_outs, sh


def _run(in_maps):
    if "nc" not in _cache:
        _cache["nc"] = _build()
    if "runner" not in _cache:
        _cache["runner"] = _make_runner(_cache["nc"])
    sharded, in_names, out_names, zero_outs, sh = _cache["runner"]
    n = NC
    concat_in = [
        jax.device_put(
            np.concatenate([np.asarray(in_maps[c][nm]) for c in range(n)], 0), sh)
        for nm in in_names
    ]
    zeros = [
        jax.device_put(np.zeros((n * z.shape[0], *z.shape[1:]), z.dtype), sh)
        for z in zero_outs
    ]
    outs = sharded(*concat_in, *zeros)
    i = out_names.index("out")
    arr = np.asarray(outs[i])           # [NC*S, DM]
    return arr.reshape(n, S, DM)


E4 = ml_dtypes.float8_e4m3


def _hilo(x):
    h = x.astype(E4)
    l = (x - h.astype(np.float32)).astype(E4)
    return np.ascontiguousarray(h), np.ascontiguousarray(l)


def kernel(q, k, v, Wq, Wk, Wv, Wo):
    q = np.asarray(q, dtype=np.float32)
    k = np.asarray(k, dtype=np.float32)
    v = np.asarray(v, dtype=np.float32)
    qT = [_hilo(q[b].T) for b in range(B)]
    kT = [_hilo(k[b].T) for b in range(B)]
    vT = [_hilo(v[b].T) for b in range(B)]
    Wq64 = np.asarray(Wq, dtype=np.float32) * WS
    Wk64 = np.asarray(Wk, dtype=np.float32) * WS
    Wv64 = np.asarray(Wv, dtype=np.float32) * WS
    Wo6 = np.asarray(Wo, dtype=np.float32).astype(np.float16)

    in_maps = []
    for c in range(NC):
        b, g = c // 4, c % 4
        wqh, wql = _hilo(Wq64[:, g * DQ:(g + 1) * DQ])
        wkh, wkl = _hilo(Wk64[:, g * 2 * DH:(g + 1) * 2 * DH])
        wvh, wvl = _hilo(Wv64[:, g * 2 * DH:(g + 1) * 2 * DH])
        in_maps.append({
            "qTh": qT[b][0], "qTl": qT[b][1],
            "kTh": kT[b][0], "kTl": kT[b][1],
            "vTh": vT[b][0], "vTl": vT[b][1],
            "wqh": wqh, "wql": wql, "wkh": wkh, "wkl": wkl,
            "wvh": wvh, "wvl": wvl,
            "wo": np.ascontiguousarray(Wo6[g * DQ:(g + 1) * DQ, :]),
        })
    partials = _run(in_maps)            # [8, S, DM]
    out = partials.astype(np.float32, copy=False).reshape(B, 4, S, DM).sum(1)
    return out
